# revision 5
# baseline (speedup 1.0000x reference)
"""Trainium2 Bass kernel for nn_BPRMF (segment_reduce): gather -> running-mean
-> BatchNorm(train) -> LIF spiking recurrence -> scores matmul.

Sharding over 8 NeuronCores:
  - gather/cumsum/BN/LIF: data-parallel over batch (64 rows/core); BN batch
    stats via AllReduce, LIF output via AllGather.
  - scores matmul + output: vocab-sharded (12800 item columns/core).

Self-contained: hardcodes shapes, builds/compiles the Bass program on first
call, caches it for the process lifetime.
"""
import sys

sys.path.insert(0, "/opt/trn_rl_repo")

import numpy as np
import ml_dtypes

N_ITEMS = 100001
D = 128
T = 50
B = 512
NCORES = 8
BSH = B // NCORES          # 64 batch rows per core
VSH = 12800                # vocab shard per core (8*12800 = 102400 >= 100001)
TH = T // 2                # 25: gather packs two time-halves on 128 partitions
TAU = 2.0
V_TH = 1.0
BN_EPS = 1e-5

_CACHE = {}
LAST_EXEC_NS = None
LAST_RESULTS = None


def _emit_iteration(nc, tc, aps, collectives=True, it=0):
    """Emit one full pipeline iteration. All pools are scoped to the call so
    an unrolled timing build reuses the same on-chip space serially."""
    import concourse.bass as bass
    from concourse import mybir
    from contextlib import ExitStack

    f32 = mybir.dt.float32
    bf16 = mybir.dt.bfloat16
    i32 = mybir.dt.int32
    Alu = mybir.AluOpType
    Act = mybir.ActivationFunctionType

    emb, embT, offs, rdiag, pp, out = (aps["emb"], aps["embT"], aps["offs"],
                                       aps["rdiag"], aps["pp"], aps["out"])
    groups = [list(range(NCORES))]

    with ExitStack() as ctx:
        con = ctx.enter_context(tc.tile_pool(name=f"con{it}", bufs=1))
        work = ctx.enter_context(tc.tile_pool(name=f"work{it}", bufs=1))
        hpool = ctx.enter_context(tc.tile_pool(name=f"hp{it}", bufs=4))
        dr = ctx.enter_context(tc.tile_pool(name=f"dr{it}", bufs=1, space="DRAM"))

        # ---- constant-ish loads ----
        offs_s = con.tile([128, TH], i32, name=f"offs_s{it}")
        nc.sync.dma_start(offs_s[:], offs)
        rdiag_s = con.tile([BSH, T * BSH], f32, name=f"rdiag_s{it}")
        nc.sync.dma_start(rdiag_s[:], rdiag)
        pp_s = con.tile([D, 2], f32, name=f"pp_s{it}")
        nc.sync.dma_start(pp_s[:], pp)
        eps_t = con.tile([D, 1], f32, name=f"eps_t{it}")
        nc.vector.memset(eps_t[:], BN_EPS)
        embT_s = con.tile([D, VSH], bf16, name=f"embT_s{it}")
        for q in range(4):
            nc.sync.dma_start(embT_s[:, q * (VSH // 4):(q + 1) * (VSH // 4)],
                              embT[:, q * (VSH // 4):(q + 1) * (VSH // 4)])

        # ---- gather: partition p<64 holds (b=p, t=j); p>=64 (b=p-64, t=j+25)
        G = con.tile([128, TH * D], f32, name=f"G{it}")
        for j in range(TH):
            nc.gpsimd.indirect_dma_start(
                out=G[:, j * D:(j + 1) * D], out_offset=None, in_=emb,
                in_offset=bass.IndirectOffsetOnAxis(ap=offs_s[:, j:j + 1], axis=0),
            )
        # HW tensor ops need operands at the same start partition: move the
        # upper 64 partitions (time steps 25..49) down to partitions 0..63.
        G2 = con.tile([BSH, TH * D], f32, name=f"G2{it}")
        nc.sync.dma_start(G2[:], G[BSH:128, :])

        with tc.tile_pool(name=f"psA{it}", bufs=1, space="PSUM") as psA:
            uFT = psA.tile([128, T * BSH], f32, name=f"uFT{it}")

            # ---- cumsum over t (DVE) + fused transpose-and-scale (PE) ----
            pf = work.tile([BSH, D], f32, name=f"pf{it}")
            for t in range(T):
                if t < TH:
                    src = G[0:BSH, t * D:(t + 1) * D]
                else:
                    src = G2[0:BSH, (t - TH) * D:(t - TH + 1) * D]
                if t == 0:
                    nc.vector.tensor_copy(pf[:], src)
                else:
                    nc.vector.tensor_tensor(out=pf[:], in0=pf[:], in1=src,
                                            op=Alu.add)
                nc.tensor.matmul(uFT[:, t * BSH:(t + 1) * BSH], lhsT=pf[:],
                                 rhs=rdiag_s[:, t * BSH:(t + 1) * BSH],
                                 start=True, stop=True)

            # ---- batch-norm stats (partial over local 64 rows) ----
            packed = work.tile([D, 2 * T], f32, name=f"packed{it}")
            uFT_v = uFT[:].rearrange("p (t b) -> p t b", t=T)
            nc.vector.tensor_reduce(out=packed[:, 0:T], in_=uFT_v,
                                    axis=mybir.AxisListType.X, op=Alu.add)
            sq = con.tile([128, T * BSH], f32, name=f"sq{it}")
            nc.scalar.activation(sq[:], uFT[:], Act.Square)
            sq_v = sq[:].rearrange("p (t b) -> p t b", t=T)
            nc.vector.tensor_reduce(out=packed[:, T:2 * T], in_=sq_v,
                                    axis=mybir.AxisListType.X, op=Alu.add)

            cc_in = dr.tile([D, 2 * T], f32, name=f"cc_in{it}")
            cc_out = dr.tile([D, 2 * T], f32, addr_space="Shared",
                             name=f"cc_out{it}")
            nc.sync.dma_start(cc_in[:], packed[:])
            if collectives:
                nc.gpsimd.collective_compute(
                    "AllReduce", Alu.add, replica_groups=groups,
                    ins=[cc_in[:]], outs=[cc_out[:]],
                )
            else:
                nc.sync.dma_start(cc_out[:], cc_in[:])
            gstats = work.tile([D, 2 * T], f32, name=f"gstats{it}")
            nc.sync.dma_start(gstats[:], cc_out[:])

            # ---- BN affine params: h_t = x*s2_t + b2_t  (pre-divided by TAU)
            mean = work.tile([D, T], f32, name=f"mean{it}")
            nc.vector.tensor_scalar(out=mean[:], in0=gstats[:, 0:T],
                                    scalar1=1.0 / B, scalar2=None, op0=Alu.mult)
            ex2 = work.tile([D, T], f32, name=f"ex2{it}")
            nc.vector.tensor_scalar(out=ex2[:], in0=gstats[:, T:2 * T],
                                    scalar1=1.0 / B, scalar2=None, op0=Alu.mult)
            var = work.tile([D, T], f32, name=f"var{it}")
            nc.vector.tensor_tensor(out=var[:], in0=mean[:], in1=mean[:],
                                    op=Alu.mult)
            nc.vector.tensor_tensor(out=var[:], in0=ex2[:], in1=var[:],
                                    op=Alu.subtract)
            std = work.tile([D, T], f32, name=f"std{it}")
            nc.scalar.activation(std[:], var[:], Act.Sqrt, bias=eps_t[:, 0:1])
            inv = work.tile([D, T], f32, name=f"inv{it}")
            nc.vector.reciprocal(inv[:], std[:])
            s2 = work.tile([D, T], f32, name=f"s2{it}")
            nc.vector.tensor_scalar(out=s2[:], in0=inv[:], scalar1=pp_s[:, 0:1],
                                    scalar2=1.0 / TAU, op0=Alu.mult, op1=Alu.mult)
            bh = work.tile([D, 1], f32, name=f"bh{it}")
            nc.vector.tensor_scalar(out=bh[:], in0=pp_s[:, 1:2],
                                    scalar1=1.0 / TAU, scalar2=None, op0=Alu.mult)
            ms = work.tile([D, T], f32, name=f"ms{it}")
            nc.vector.tensor_tensor(out=ms[:], in0=mean[:], in1=s2[:], op=Alu.mult)
            b2 = work.tile([D, T], f32, name=f"b2{it}")
            nc.vector.scalar_tensor_tensor(
                out=b2[:], in0=ms[:], scalar=-1.0,
                in1=bh[:, 0:1].to_broadcast((D, T)), op0=Alu.mult, op1=Alu.add)

            # ---- LIF recurrence (v = v/2 + h; spike; soft reset) ----
            v = work.tile([128, BSH], f32, name=f"v{it}")
            nc.vector.memset(v[:], 0.0)
            spk = con.tile([128, T * BSH], f32, name=f"spk{it}")
            for t in range(T):
                h = hpool.tile([128, BSH], f32, tag="h", name=f"h{it}_{t}")
                nc.scalar.activation(h[:], uFT[:, t * BSH:(t + 1) * BSH],
                                     Act.Identity, scale=s2[:, t:t + 1],
                                     bias=b2[:, t:t + 1])
                nc.vector.scalar_tensor_tensor(out=v[:], in0=v[:], scalar=1.0 / TAU,
                                               in1=h[:], op0=Alu.mult, op1=Alu.add)
                spk_t = spk[:, t * BSH:(t + 1) * BSH]
                nc.vector.tensor_scalar(out=spk_t, in0=v[:], scalar1=V_TH,
                                        scalar2=None, op0=Alu.is_ge)
                nc.vector.tensor_tensor(out=v[:], in0=v[:], in1=spk_t,
                                        op=Alu.subtract)

            acc = work.tile([128, BSH], f32, name=f"acc{it}")
            spk_v = spk[:].rearrange("p (t b) -> p b t", t=T)
            nc.vector.tensor_reduce(out=acc[:], in_=spk_v,
                                    axis=mybir.AxisListType.X, op=Alu.add)
            uo = work.tile([128, BSH], bf16, name=f"uo{it}")
            nc.vector.tensor_scalar(out=uo[:], in0=acc[:], scalar1=1.0 / T,
                                    scalar2=None, op0=Alu.mult)

        # ---- AllGather uF_out^T -> lhsT [128, 512] (bf16) ----
        ag_in = dr.tile([D, BSH], bf16, name=f"ag_in{it}")
        ag_out = dr.tile([NCORES * D, BSH], bf16, addr_space="Shared",
                         name=f"ag_out{it}")
        nc.sync.dma_start(ag_in[:], uo[:])
        if collectives:
            nc.gpsimd.collective_compute(
                "AllGather", Alu.bypass, replica_groups=groups,
                ins=[ag_in[:]], outs=[ag_out[:]],
            )
        lhsT = con.tile([D, B], bf16, name=f"lhsT{it}")
        for c in range(NCORES):
            if collectives:
                src = ag_out[c * D:(c + 1) * D, :]
            else:
                src = ag_in[:]
            nc.sync.dma_start(lhsT[:, c * BSH:(c + 1) * BSH], src)

        # ---- scores matmul, vocab-sharded ----
        NBLK = 512
        with tc.tile_pool(name=f"psB{it}", bufs=8, space="PSUM") as psB, \
             tc.tile_pool(name=f"ost{it}", bufs=6) as ostage:
            k = 0
            for m in range(B // 128):
                for n in range(VSH // NBLK):
                    mm = psB.tile([128, NBLK], f32, tag="mm", name=f"mm{it}_{k}")
                    nc.tensor.matmul(mm[:], lhsT=lhsT[:, m * 128:(m + 1) * 128],
                                     rhs=embT_s[:, n * NBLK:(n + 1) * NBLK],
                                     start=True, stop=True)
                    ot = ostage.tile([128, NBLK], f32, tag="ot", name=f"ot{it}_{k}")
                    if k % 2 == 0:
                        nc.vector.tensor_copy(ot[:], mm[:])
                    else:
                        nc.scalar.activation(ot[:], mm[:], Act.Copy)
                    nc.sync.dma_start(
                        out[m * 128:(m + 1) * 128, n * NBLK:(n + 1) * NBLK], ot[:])
                    k += 1


def _build(unroll=1, collectives=True, num_devices=NCORES):
    import concourse.tile as tile
    from concourse import bacc, mybir

    f32 = mybir.dt.float32
    bf16 = mybir.dt.bfloat16
    i32 = mybir.dt.int32

    nc = bacc.Bacc("TRN2", target_bir_lowering=False, debug=False,
                   num_devices=num_devices)
    aps = {
        "emb": nc.dram_tensor("emb", [N_ITEMS, D], f32, kind="ExternalInput").ap(),
        "embT": nc.dram_tensor("embT", [D, VSH], bf16, kind="ExternalInput").ap(),
        "offs": nc.dram_tensor("offs", [128, TH], i32, kind="ExternalInput").ap(),
        "rdiag": nc.dram_tensor("rdiag", [BSH, T * BSH], f32,
                                kind="ExternalInput").ap(),
        "pp": nc.dram_tensor("pp", [D, 2], f32, kind="ExternalInput").ap(),
        "out": nc.dram_tensor("out", [B, VSH], f32, kind="ExternalOutput").ap(),
    }
    with tile.TileContext(nc) as tc:
        for it in range(unroll):
            _emit_iteration(nc, tc, aps, collectives=collectives, it=it)
    nc.compile()
    return nc


def _prep_inputs(seq, lengths, emb_table, gamma, beta):
    seq = np.asarray(seq)
    lengths = np.asarray(lengths)
    emb_table = np.asarray(emb_table, dtype=np.float32)
    gamma = np.asarray(gamma, dtype=np.float32)
    beta = np.asarray(beta, dtype=np.float32)

    emb_full = emb_table.copy()
    emb_full[0, :] = 0.0

    tt = np.arange(1, T + 1, dtype=np.float64)[None, :]
    denom = np.minimum(tt, lengths.astype(np.float64)[:, None])
    rd = (1.0 / denom).astype(np.float32)                      # [B, T]

    embT_full = np.zeros((D, NCORES * VSH), dtype=ml_dtypes.bfloat16)
    embT_full[:, :N_ITEMS] = emb_full.T.astype(ml_dtypes.bfloat16)

    pp = np.stack([gamma, beta], axis=1).astype(np.float32)    # [128, 2]

    in_maps = []
    for c in range(NCORES):
        sl = slice(c * BSH, (c + 1) * BSH)
        seq_c = seq[sl].astype(np.int32)                       # [64, 50]
        offs_c = np.concatenate([seq_c[:, :TH], seq_c[:, TH:]], axis=0)
        offs_c = np.ascontiguousarray(offs_c)                  # [128, 25]
        rd_c = rd[sl]                                          # [64, 50]
        r3 = np.zeros((BSH, T, BSH), dtype=np.float32)
        for b in range(BSH):
            r3[b, :, b] = rd_c[b]
        rdiag_c = np.ascontiguousarray(r3.reshape(BSH, T * BSH))
        embT_c = np.ascontiguousarray(embT_full[:, c * VSH:(c + 1) * VSH])
        in_maps.append({
            "emb": emb_full, "embT": embT_c, "offs": offs_c,
            "rdiag": rdiag_c, "pp": pp,
        })
    return in_maps


def _cached_runner(nc, reps_key):
    """Build (once) a jitted shard_map runner with device-resident input
    placement for repeated timed executions of nc's single bass_exec."""
    import jax
    from jax.sharding import Mesh, PartitionSpec
    from jax.experimental.shard_map import shard_map
    from concourse import mybir
    from concourse.bass2jax import (_bass_exec_p, partition_id_tensor,
                                    install_neuronx_cc_hook)
    install_neuronx_cc_hook()

    in_names, out_names, out_avals = [], [], []
    for alloc in nc.m.functions[0].allocations:
        if not isinstance(alloc, mybir.MemoryLocationSet):
            continue
        name = alloc.memorylocations[0].name
        if alloc.kind == "ExternalInput":
            if nc.partition_id_tensor is None or name != nc.partition_id_tensor.name:
                in_names.append(name)
        elif alloc.kind == "ExternalOutput":
            out_names.append(name)
            out_avals.append(jax.core.ShapedArray(
                tuple(alloc.tensor_shape), mybir.dt.np(alloc.dtype)))
    n_params = len(in_names)
    all_in = list(in_names) + list(out_names)
    if nc.partition_id_tensor is not None:
        all_in.append(nc.partition_id_tensor.name)

    def _body(*args):
        operands = list(args)
        if nc.partition_id_tensor is not None:
            operands.append(partition_id_tensor())
        return tuple(_bass_exec_p.bind(
            *operands, out_avals=tuple(out_avals), in_names=tuple(all_in),
            out_names=tuple(out_names), lowering_input_output_aliases=(),
            sim_require_finite=True, sim_require_nnan=True, nc=nc))

    mesh = Mesh(np.asarray(jax.devices()[:NCORES]), ("core",))
    n_outs = len(out_names)
    f = jax.jit(shard_map(
        _body, mesh=mesh,
        in_specs=(PartitionSpec("core"),) * (n_params + n_outs),
        out_specs=(PartitionSpec("core"),) * n_outs, check_rep=False))
    return f, in_names, out_avals


def _timed(nc, in_maps, reps=16):
    import jax, time
    f, in_names, out_avals = _cached_runner(nc, None)
    per_core = [[np.asarray(m[nm]) for nm in in_names] for m in in_maps]
    ci = [jax.device_put(np.concatenate([per_core[c][i] for c in range(NCORES)],
                                        axis=0)) for i in range(len(in_names))]
    cz = [jax.device_put(np.zeros((NCORES * a.shape[0], *a.shape[1:]), a.dtype))
          for a in out_avals]
    out = f(*ci, *cz)
    jax.block_until_ready(out)
    ts = []
    for _ in range(reps):
        t0 = time.perf_counter()
        out = f(*ci, *cz)
        jax.block_until_ready(out)
        ts.append(time.perf_counter() - t0)
    return ts


def benchmark(seq, lengths, emb_table, gamma, beta, unroll=8, reps=16):
    """Estimate per-iteration device time via the slope between a 1x and an
    unrolled Kx build of the same program (identical I/O staging costs)."""
    in_maps = _prep_inputs(seq, lengths, emb_table, gamma, beta)
    if "nc" not in _CACHE:
        _CACHE["nc"] = _build()
    key = f"nc{unroll}"
    if key not in _CACHE:
        _CACHE[key] = _build(unroll=unroll)
    t1 = _timed(_CACHE["nc"], in_maps, reps=reps)
    tk = _timed(_CACHE[key], in_maps, reps=reps)
    t1s, tks = sorted(t1), sorted(tk)
    med1 = t1s[len(t1s) // 2]
    medk = tks[len(tks) // 2]
    per_iter_ns = (medk - med1) / (unroll - 1) * 1e9
    return per_iter_ns, {"t1_med_ms": med1 * 1e3, "tk_med_ms": medk * 1e3,
                         "t1_min_ms": min(t1) * 1e3, "tk_min_ms": min(tk) * 1e3,
                         "slope_min_ns": (min(tk) - min(t1)) / (unroll - 1) * 1e9}


def kernel(seq, lengths, emb_table, gamma, beta, trace=False):
    global LAST_EXEC_NS, LAST_RESULTS
    from concourse.bass_utils import run_bass_kernel_spmd

    if "nc" not in _CACHE:
        _CACHE["nc"] = _build()
    nc = _CACHE["nc"]

    in_maps = _prep_inputs(seq, lengths, emb_table, gamma, beta)
    res = run_bass_kernel_spmd(nc, in_maps, core_ids=list(range(NCORES)))
    LAST_EXEC_NS = res.exec_time_ns
    LAST_RESULTS = res
    scores = np.concatenate([res.results[c]["out"] for c in range(NCORES)],
                            axis=1)[:, :N_ITEMS]
    return np.ascontiguousarray(scores.astype(np.float32))


# revision 14
# speedup vs baseline: 1.2913x; 1.2913x over previous
"""Trainium2 Bass kernel for nn_BPRMF (segment_reduce): gather -> running-mean
-> BatchNorm(train) -> LIF spiking recurrence -> scores matmul.

Sharding over 8 NeuronCores:
  - gather/cumsum/BN/LIF: data-parallel over batch (64 rows/core); BN batch
    stats via AllReduce, LIF output via AllGather.
  - scores matmul + output: vocab-sharded (12800 item columns/core).

Self-contained: hardcodes shapes, builds/compiles the Bass program on first
call, caches it for the process lifetime.
"""
import sys

sys.path.insert(0, "/opt/trn_rl_repo")

import numpy as np
import ml_dtypes

N_ITEMS = 100001
D = 128
T = 50
B = 512
NCORES = 8
BSH = B // NCORES          # 64 batch rows per core
VSH = 12800                # vocab shard per core (8*12800 = 102400 >= 100001)
TH = T // 2                # 25: gather packs two time-halves on 128 partitions
TAU = 2.0
V_TH = 1.0
BN_EPS = 1e-5

_CACHE = {}
LAST_EXEC_NS = None
LAST_RESULTS = None


def _emit_iteration(nc, tc, aps, collectives=True, it=0):
    """Emit one full pipeline iteration. All pools are scoped to the call so
    an unrolled timing build reuses the same on-chip space serially."""
    import concourse.bass as bass
    from concourse import mybir
    from contextlib import ExitStack

    f32 = mybir.dt.float32
    bf16 = mybir.dt.bfloat16
    i32 = mybir.dt.int32
    Alu = mybir.AluOpType
    Act = mybir.ActivationFunctionType

    emb, embT, offs, rdiag, pp, out = (aps["emb"], aps["embT"], aps["offs"],
                                       aps["rdiag"], aps["pp"], aps["out"])
    groups = [list(range(NCORES))]

    with ExitStack() as ctx:
        con = ctx.enter_context(tc.tile_pool(name=f"con{it}", bufs=1))
        work = ctx.enter_context(tc.tile_pool(name=f"work{it}", bufs=1))
        hpool = ctx.enter_context(tc.tile_pool(name=f"hp{it}", bufs=4))
        dr = ctx.enter_context(tc.tile_pool(name=f"dr{it}", bufs=1, space="DRAM"))

        # ---- constant-ish loads ----
        offs_s = con.tile([128, TH], i32, name=f"offs_s{it}")
        nc.sync.dma_start(offs_s[:], offs)
        rdiag_s = con.tile([BSH, T * BSH], f32, name=f"rdiag_s{it}")
        nc.sync.dma_start(rdiag_s[:], rdiag)
        pp_s = con.tile([D, 2], f32, name=f"pp_s{it}")
        nc.sync.dma_start(pp_s[:], pp)
        eps_t = con.tile([D, 1], f32, name=f"eps_t{it}")
        nc.vector.memset(eps_t[:], BN_EPS)
        embT_s = con.tile([D, VSH], bf16, name=f"embT_s{it}")
        for q in range(4):
            nc.sync.dma_start(embT_s[:, q * (VSH // 4):(q + 1) * (VSH // 4)],
                              embT[:, q * (VSH // 4):(q + 1) * (VSH // 4)])

        # ---- gather: partition p<64 holds (b=p, t=j); p>=64 (b=p-64, t=j+25)
        # HW tensor ops need operands at the same start partition, so the
        # upper 64 partitions (time steps 25..49) are copied down to G2 in
        # chunks pipelined with the gather stream.
        G = con.tile([128, TH * D], f32, name=f"G{it}")
        G2 = con.tile([BSH, TH * D], f32, name=f"G2{it}")
        GCH = 5
        for j in range(TH):
            nc.gpsimd.indirect_dma_start(
                out=G[:, j * D:(j + 1) * D], out_offset=None, in_=emb,
                in_offset=bass.IndirectOffsetOnAxis(ap=offs_s[:, j:j + 1], axis=0),
            )
            if (j + 1) % GCH == 0:
                lo = (j + 1 - GCH) * D
                hi = (j + 1) * D
                nc.sync.dma_start(G2[:, lo:hi], G[BSH:128, lo:hi])

        with tc.tile_pool(name=f"psA{it}", bufs=1, space="PSUM") as psA:
            uFT = psA.tile([128, T * BSH], f32, name=f"uFT{it}")

            # ---- cumsum over t (DVE) + fused transpose-and-scale (PE), with
            # BN partial stats pipelined per completed PSUM bank (8 t-slices)
            # so stats reads never touch the bank PE is currently writing.
            TPB = 8  # 512 f32 bank / 64-wide slices
            # ping-pong prefix buffers: the transpose of step t reads pf[t%2]
            # while the DVE cumsum for t+1 writes pf[(t+1)%2] (no WAR stall)
            pf0 = work.tile([BSH, D], f32, name=f"pf0{it}")
            pf1 = work.tile([BSH, D], f32, name=f"pf1{it}")
            pfs = [pf0, pf1]
            packed = work.tile([D, 2 * T], f32, name=f"packed{it}")

            def stats_chunk(t0, t1):
                n = t1 - t0
                xs = uFT[:, t0 * BSH:t1 * BSH]
                nc.vector.tensor_reduce(
                    out=packed[:, t0:t1], in_=xs.rearrange("p (t b) -> p t b", t=n),
                    axis=mybir.AxisListType.X, op=Alu.add)
                sqb = hpool.tile([128, TPB * BSH], f32, tag="sqb",
                                 name=f"sqb{it}_{t0}")
                nc.scalar.activation(sqb[:, 0:n * BSH], xs, Act.Square)
                nc.vector.tensor_reduce(
                    out=packed[:, T + t0:T + t1],
                    in_=sqb[:, 0:n * BSH].rearrange("p (t b) -> p t b", t=n),
                    axis=mybir.AxisListType.X, op=Alu.add)

            for t in range(T):
                if t < TH:
                    src = G[0:BSH, t * D:(t + 1) * D]
                else:
                    src = G2[0:BSH, (t - TH) * D:(t - TH + 1) * D]
                pf = pfs[t % 2]
                if t == 0:
                    nc.vector.tensor_copy(pf[:], src)
                else:
                    nc.vector.tensor_tensor(out=pf[:], in0=pfs[(t - 1) % 2][:],
                                            in1=src, op=Alu.add)
                nc.tensor.matmul(uFT[:, t * BSH:(t + 1) * BSH], lhsT=pf[:],
                                 rhs=rdiag_s[:, t * BSH:(t + 1) * BSH],
                                 start=True, stop=True)
                if t % TPB == TPB - 1:
                    stats_chunk(t - TPB + 1, t + 1)
            if T % TPB:
                stats_chunk(T - T % TPB, T)

            cc_in = dr.tile([D, 2 * T], f32, name=f"cc_in{it}")
            cc_out = dr.tile([D, 2 * T], f32, addr_space="Shared",
                             name=f"cc_out{it}")
            nc.sync.dma_start(cc_in[:], packed[:])
            if collectives:
                nc.gpsimd.collective_compute(
                    "AllReduce", Alu.add, replica_groups=groups,
                    ins=[cc_in[:]], outs=[cc_out[:]],
                )
            else:
                nc.sync.dma_start(cc_out[:], cc_in[:])
            gstats = work.tile([D, 2 * T], f32, name=f"gstats{it}")
            nc.sync.dma_start(gstats[:], cc_out[:])

            # ---- BN affine params: h_t = x*s2_t + b2_t  (pre-divided by TAU)
            mean = work.tile([D, T], f32, name=f"mean{it}")
            nc.vector.tensor_scalar(out=mean[:], in0=gstats[:, 0:T],
                                    scalar1=1.0 / B, scalar2=None, op0=Alu.mult)
            ex2 = work.tile([D, T], f32, name=f"ex2{it}")
            nc.vector.tensor_scalar(out=ex2[:], in0=gstats[:, T:2 * T],
                                    scalar1=1.0 / B, scalar2=None, op0=Alu.mult)
            var = work.tile([D, T], f32, name=f"var{it}")
            nc.vector.tensor_tensor(out=var[:], in0=mean[:], in1=mean[:],
                                    op=Alu.mult)
            nc.vector.tensor_tensor(out=var[:], in0=ex2[:], in1=var[:],
                                    op=Alu.subtract)
            std = work.tile([D, T], f32, name=f"std{it}")
            nc.scalar.activation(std[:], var[:], Act.Sqrt, bias=eps_t[:, 0:1])
            inv = work.tile([D, T], f32, name=f"inv{it}")
            nc.vector.reciprocal(inv[:], std[:])
            s2 = work.tile([D, T], f32, name=f"s2{it}")
            nc.vector.tensor_scalar(out=s2[:], in0=inv[:], scalar1=pp_s[:, 0:1],
                                    scalar2=1.0 / TAU, op0=Alu.mult, op1=Alu.mult)
            bh = work.tile([D, 1], f32, name=f"bh{it}")
            nc.vector.tensor_scalar(out=bh[:], in0=pp_s[:, 1:2],
                                    scalar1=1.0 / TAU, scalar2=None, op0=Alu.mult)
            ms = work.tile([D, T], f32, name=f"ms{it}")
            nc.vector.tensor_tensor(out=ms[:], in0=mean[:], in1=s2[:], op=Alu.mult)
            b2 = work.tile([D, T], f32, name=f"b2{it}")
            nc.vector.scalar_tensor_tensor(
                out=b2[:], in0=ms[:], scalar=-1.0,
                in1=bh[:, 0:1].to_broadcast((D, T)), op0=Alu.mult, op1=Alu.add)

            # ---- LIF recurrence on the pre-reset voltage w:
            #   s_t = [w_t >= 1];  w_{t+1} = (w_t - s_t)/2 + h_{t+1}
            # evaluated as q = w/2 + h (indep of s) then w' = q - s/2, so each
            # DVE op only depends on the immediately preceding ones (no stalls).
            w = work.tile([128, BSH], f32, name=f"w{it}")
            q = work.tile([128, BSH], f32, name=f"q{it}")
            spk = con.tile([128, T * BSH], f32, name=f"spk{it}")
            for t in range(T):
                h = hpool.tile([128, BSH], f32, tag="h", name=f"h{it}_{t}")
                nc.scalar.activation(h[:], uFT[:, t * BSH:(t + 1) * BSH],
                                     Act.Identity, scale=s2[:, t:t + 1],
                                     bias=b2[:, t:t + 1])
                if t == 0:
                    nc.vector.tensor_copy(w[:], h[:])
                else:
                    nc.vector.scalar_tensor_tensor(
                        out=q[:], in0=w[:], scalar=1.0 / TAU, in1=h[:],
                        op0=Alu.mult, op1=Alu.add)
                    nc.vector.scalar_tensor_tensor(
                        out=w[:], in0=spk[:, (t - 1) * BSH:t * BSH],
                        scalar=-V_TH / TAU, in1=q[:], op0=Alu.mult, op1=Alu.add)
                nc.vector.tensor_scalar(out=spk[:, t * BSH:(t + 1) * BSH],
                                        in0=w[:], scalar1=V_TH,
                                        scalar2=None, op0=Alu.is_ge)

            acc = work.tile([128, BSH], f32, name=f"acc{it}")
            spk_v = spk[:].rearrange("p (t b) -> p b t", t=T)
            nc.vector.tensor_reduce(out=acc[:], in_=spk_v,
                                    axis=mybir.AxisListType.X, op=Alu.add)
            uo = work.tile([128, BSH], bf16, name=f"uo{it}")
            nc.vector.tensor_scalar(out=uo[:], in0=acc[:], scalar1=1.0 / T,
                                    scalar2=None, op0=Alu.mult)

        # ---- AllGather uF_out^T -> lhsT [128, 512] (bf16) ----
        ag_in = dr.tile([D, BSH], bf16, name=f"ag_in{it}")
        ag_out = dr.tile([NCORES * D, BSH], bf16, addr_space="Shared",
                         name=f"ag_out{it}")
        nc.sync.dma_start(ag_in[:], uo[:])
        if collectives:
            nc.gpsimd.collective_compute(
                "AllGather", Alu.bypass, replica_groups=groups,
                ins=[ag_in[:]], outs=[ag_out[:]],
            )
        lhsT = con.tile([D, B], bf16, name=f"lhsT{it}")
        for c in range(NCORES):
            if collectives:
                src = ag_out[c * D:(c + 1) * D, :]
            else:
                src = ag_in[:]
            nc.sync.dma_start(lhsT[:, c * BSH:(c + 1) * BSH], src)

        # ---- scores matmul, vocab-sharded ----
        # Evict 4 psum blocks into one wide staging tile per out-DMA so each
        # partition row sends 4KB contiguous (HWDGE descriptor-gen bound
        # otherwise).
        NBLK = 512
        GRP = 4
        with tc.tile_pool(name=f"psB{it}", bufs=8, space="PSUM") as psB, \
             tc.tile_pool(name=f"ost{it}", bufs=4) as ostage:
            k = 0
            for m in range(B // 128):
                n = 0
                while n < VSH // NBLK:
                    g = min(GRP, VSH // NBLK - n)
                    ot = ostage.tile([128, GRP * NBLK], bf16, tag="ot",
                                     name=f"ot{it}_{m}_{n}")
                    for i in range(g):
                        mm = psB.tile([128, NBLK], f32, tag="mm",
                                      name=f"mm{it}_{k}")
                        nc.tensor.matmul(
                            mm[:], lhsT=lhsT[:, m * 128:(m + 1) * 128],
                            rhs=embT_s[:, (n + i) * NBLK:(n + i + 1) * NBLK],
                            start=True, stop=True)
                        dst = ot[:, i * NBLK:(i + 1) * NBLK]
                        if k % 2 == 0:
                            nc.vector.tensor_copy(dst, mm[:])
                        else:
                            nc.scalar.activation(dst, mm[:], Act.Copy)
                        k += 1
                    nc.sync.dma_start(
                        out[m * 128:(m + 1) * 128,
                            n * NBLK:(n + g) * NBLK], ot[:, 0:g * NBLK])
                    n += g


def _build(unroll=1, collectives=True, num_devices=NCORES):
    import concourse.tile as tile
    from concourse import bacc, mybir

    f32 = mybir.dt.float32
    bf16 = mybir.dt.bfloat16
    i32 = mybir.dt.int32

    nc = bacc.Bacc("TRN2", target_bir_lowering=False, debug=False,
                   num_devices=num_devices)
    aps = {
        "emb": nc.dram_tensor("emb", [N_ITEMS, D], f32, kind="ExternalInput").ap(),
        "embT": nc.dram_tensor("embT", [D, VSH], bf16, kind="ExternalInput").ap(),
        "offs": nc.dram_tensor("offs", [128, TH], i32, kind="ExternalInput").ap(),
        "rdiag": nc.dram_tensor("rdiag", [BSH, T * BSH], f32,
                                kind="ExternalInput").ap(),
        "pp": nc.dram_tensor("pp", [D, 2], f32, kind="ExternalInput").ap(),
        "out": nc.dram_tensor("out", [B, VSH], bf16, kind="ExternalOutput").ap(),
    }
    with tile.TileContext(nc) as tc:
        for it in range(unroll):
            _emit_iteration(nc, tc, aps, collectives=collectives, it=it)
    nc.compile()
    return nc


def _prep_inputs(seq, lengths, emb_table, gamma, beta):
    seq = np.asarray(seq)
    lengths = np.asarray(lengths)
    emb_table = np.asarray(emb_table, dtype=np.float32)
    gamma = np.asarray(gamma, dtype=np.float32)
    beta = np.asarray(beta, dtype=np.float32)

    emb_full = emb_table.copy()
    emb_full[0, :] = 0.0

    tt = np.arange(1, T + 1, dtype=np.float64)[None, :]
    denom = np.minimum(tt, lengths.astype(np.float64)[:, None])
    rd = (1.0 / denom).astype(np.float32)                      # [B, T]

    embT_full = np.zeros((D, NCORES * VSH), dtype=ml_dtypes.bfloat16)
    embT_full[:, :N_ITEMS] = emb_full.T.astype(ml_dtypes.bfloat16)

    pp = np.stack([gamma, beta], axis=1).astype(np.float32)    # [128, 2]

    in_maps = []
    for c in range(NCORES):
        sl = slice(c * BSH, (c + 1) * BSH)
        seq_c = seq[sl].astype(np.int32)                       # [64, 50]
        offs_c = np.concatenate([seq_c[:, :TH], seq_c[:, TH:]], axis=0)
        offs_c = np.ascontiguousarray(offs_c)                  # [128, 25]
        rd_c = rd[sl]                                          # [64, 50]
        r3 = np.zeros((BSH, T, BSH), dtype=np.float32)
        for b in range(BSH):
            r3[b, :, b] = rd_c[b]
        rdiag_c = np.ascontiguousarray(r3.reshape(BSH, T * BSH))
        embT_c = np.ascontiguousarray(embT_full[:, c * VSH:(c + 1) * VSH])
        in_maps.append({
            "emb": emb_full, "embT": embT_c, "offs": offs_c,
            "rdiag": rdiag_c, "pp": pp,
        })
    return in_maps


def _cached_runner(nc, reps_key):
    """Build (once) a jitted shard_map runner with device-resident input
    placement for repeated timed executions of nc's single bass_exec."""
    import jax
    from jax.sharding import Mesh, PartitionSpec
    from jax.experimental.shard_map import shard_map
    from concourse import mybir
    from concourse.bass2jax import (_bass_exec_p, partition_id_tensor,
                                    install_neuronx_cc_hook)
    install_neuronx_cc_hook()

    in_names, out_names, out_avals = [], [], []
    for alloc in nc.m.functions[0].allocations:
        if not isinstance(alloc, mybir.MemoryLocationSet):
            continue
        name = alloc.memorylocations[0].name
        if alloc.kind == "ExternalInput":
            if nc.partition_id_tensor is None or name != nc.partition_id_tensor.name:
                in_names.append(name)
        elif alloc.kind == "ExternalOutput":
            out_names.append(name)
            out_avals.append(jax.core.ShapedArray(
                tuple(alloc.tensor_shape), mybir.dt.np(alloc.dtype)))
    n_params = len(in_names)
    all_in = list(in_names) + list(out_names)
    if nc.partition_id_tensor is not None:
        all_in.append(nc.partition_id_tensor.name)

    def _body(*args):
        operands = list(args)
        if nc.partition_id_tensor is not None:
            operands.append(partition_id_tensor())
        return tuple(_bass_exec_p.bind(
            *operands, out_avals=tuple(out_avals), in_names=tuple(all_in),
            out_names=tuple(out_names), lowering_input_output_aliases=(),
            sim_require_finite=True, sim_require_nnan=True, nc=nc))

    mesh = Mesh(np.asarray(jax.devices()[:NCORES]), ("core",))
    n_outs = len(out_names)
    f = jax.jit(shard_map(
        _body, mesh=mesh,
        in_specs=(PartitionSpec("core"),) * (n_params + n_outs),
        out_specs=(PartitionSpec("core"),) * n_outs, check_rep=False))
    return f, in_names, out_avals


def _timed(nc, in_maps, reps=16):
    import jax, time
    f, in_names, out_avals = _cached_runner(nc, None)
    per_core = [[np.asarray(m[nm]) for nm in in_names] for m in in_maps]
    ci = [jax.device_put(np.concatenate([per_core[c][i] for c in range(NCORES)],
                                        axis=0)) for i in range(len(in_names))]
    cz = [jax.device_put(np.zeros((NCORES * a.shape[0], *a.shape[1:]), a.dtype))
          for a in out_avals]
    out = f(*ci, *cz)
    jax.block_until_ready(out)
    ts = []
    for _ in range(reps):
        t0 = time.perf_counter()
        out = f(*ci, *cz)
        jax.block_until_ready(out)
        ts.append(time.perf_counter() - t0)
    return ts


def benchmark(seq, lengths, emb_table, gamma, beta, unroll=8, reps=16):
    """Estimate per-iteration device time via the slope between a 1x and an
    unrolled Kx build of the same program (identical I/O staging costs)."""
    in_maps = _prep_inputs(seq, lengths, emb_table, gamma, beta)
    if "nc" not in _CACHE:
        _CACHE["nc"] = _build()
    key = f"nc{unroll}"
    if key not in _CACHE:
        _CACHE[key] = _build(unroll=unroll)
    t1 = _timed(_CACHE["nc"], in_maps, reps=reps)
    tk = _timed(_CACHE[key], in_maps, reps=reps)
    t1s, tks = sorted(t1), sorted(tk)
    med1 = t1s[len(t1s) // 2]
    medk = tks[len(tks) // 2]
    per_iter_ns = (medk - med1) / (unroll - 1) * 1e9
    return per_iter_ns, {"t1_med_ms": med1 * 1e3, "tk_med_ms": medk * 1e3,
                         "t1_min_ms": min(t1) * 1e3, "tk_min_ms": min(tk) * 1e3,
                         "slope_min_ns": (min(tk) - min(t1)) / (unroll - 1) * 1e9}


def kernel(seq, lengths, emb_table, gamma, beta, trace=False):
    global LAST_EXEC_NS, LAST_RESULTS
    from concourse.bass_utils import run_bass_kernel_spmd

    if "nc" not in _CACHE:
        _CACHE["nc"] = _build()
    nc = _CACHE["nc"]

    in_maps = _prep_inputs(seq, lengths, emb_table, gamma, beta)
    res = run_bass_kernel_spmd(nc, in_maps, core_ids=list(range(NCORES)))
    LAST_EXEC_NS = res.exec_time_ns
    LAST_RESULTS = res
    scores = np.concatenate([res.results[c]["out"] for c in range(NCORES)],
                            axis=1)[:, :N_ITEMS]
    return np.ascontiguousarray(scores.astype(np.float32))
